# revision 32
# baseline (speedup 1.0000x reference)
"""DiT block kernel for Trainium2, data-parallel over batch across 8 NeuronCores.

Per-core layout: all activations are kept transposed ([feature, token]) so that
every GEMM consumes weights in their native [in, out] layout as lhsT and
activations as rhs, with no on-device transposes anywhere.

v3 notes (from the v2 HW profile: 943us, PE cold 41% of the time, ~7.5us
PE+ACT stall per attention unit on the 1/Z DRAM bounce, 73us hole at the
LN2->MLP seam, MLP weights streamed twice). Measured 673us after:
  - Every partition-broadcast that used to bounce rows through DRAM (softmax
    1/Z, LN rstd) is now a ones-matmul on the PE: copy the PSUM row to SBUF,
    one K=1 matmul broadcasts it across partitions, full-width DVE reciprocal,
    and the normalize multiply reads the oacc PSUM tile directly (which also
    frees the PSUM slot). Zero DMA hops on any per-unit critical path.
  - LN gamma/beta modulation is folded into per-kt outer-product matmuls:
    A = (1+g) (x) rstd and B = (1+g) (x) (-mu*rstd) come from K=2 matmuls
    (lhsT = [g_row; ones]), so the apply is 2 DVE ops per (kt, half).
  - The attention drain for unit n is emitted after unit n+1's first scores
    matmuls so the PE FIFO never blocks the exp cadence (ACT is the bound
    engine in attention); the adaLN chunks 8-23 fill the remaining PE slack
    inside the attention phase instead of lengthening the qkT phase.
  - MLP runs single-pass over the full 1024-token width (hidden [128,32,1024]
    in SBUF) so wmlp1/wmlp2 stream once, not twice.
  - x and x1 stay resident in SBUF as bf16; no x re-read, no x1 DRAM round
    trip. LN rstd comes from exp(-0.5*ln(var+eps)) on the ACT engine (this
    walrus has no fast reciprocal path; DVE reciprocal costs ~6.5ns/elem, so
    the only one left runs once per attention unit on a partition-packed
    [128,512] tile). Attention is exp-bound (ACT ~100%) and the MLP runs at
    the bf16 matmul floor.

Host side shards B=8 one element per core, pre-transposes x, pre-casts weights
to bf16 (fp32 accumulation in PSUM throughout), and transposes the per-core
[D, N] outputs back.
"""
import sys

for _p in ("/opt/trn_rl_repo",):
    if _p not in sys.path:
        sys.path.insert(0, _p)

import numpy as np
import ml_dtypes
from contextlib import ExitStack

import concourse.bass as bass
import concourse.mybir as mybir
import concourse.tile as tile

f32 = mybir.dt.float32
bf16 = mybir.dt.bfloat16
AF = mybir.ActivationFunctionType
OP = mybir.AluOpType

P = 128
NTOK = 1024     # tokens per batch element
D = 1024        # model dim
KD = D // P     # 8 k-tiles over model dim
H = 16          # heads
DH = 64         # head dim
F = 4096        # mlp hidden
KF = F // P     # 32
ADA = 6 * D     # 6144
EPS = 1e-6
NCORES = 8
HALVES = (0, 512)
NCH = 256       # adaLN chunk width


def _split_multi_waits(nc):
    """This container's walrus build encodes at most ONE sync wait per
    instruction ("Too many sync wait commands"); hoist extra waits onto
    single-wait NoOps in the same engine stream."""
    for fn in nc.m.functions:
        for blk in fn.blocks:
            out = []
            for inst in blk.instructions:
                si = inst.sync_info
                waits = list(si.on_wait) if si is not None and si.on_wait else []
                if len(waits) > 1:
                    for i, w in enumerate(waits[:-1]):
                        nop = mybir.InstNoOp(name=f"{inst.name}-ws{i}", ins=[], outs=[])
                        nop.engine = inst.engine
                        nop.sync_info = mybir.SyncInfo(on_wait=[w], on_update=[])
                        out.append(nop)
                    inst.sync_info = mybir.SyncInfo(on_wait=[waits[-1]],
                                                    on_update=list(si.on_update))
                out.append(inst)
            blk.instructions = out


def build_nc(split_waits=True):
    nc = bass.Bass(trn_type="TRN2")

    xT_d = nc.dram_tensor("xT", [D, NTOK], f32, kind="ExternalInput")
    ccol_d = nc.dram_tensor("ccol", [P, KD], f32, kind="ExternalInput")
    wqk_d = nc.dram_tensor("wqk", [16, P, KD, P], bf16, kind="ExternalInput")
    wv_d = nc.dram_tensor("wv", [P, KD, D], bf16, kind="ExternalInput")
    bqk_col_d = nc.dram_tensor("bqk_col", [P, 16], f32, kind="ExternalInput")
    bv_row_d = nc.dram_tensor("bv_row", [1, D], bf16, kind="ExternalInput")
    wproj_d = nc.dram_tensor("wproj", [KD, P, KD, P], bf16, kind="ExternalInput")
    bproj_col_d = nc.dram_tensor("bproj_col", [P, KD], f32, kind="ExternalInput")
    wmlp1_d = nc.dram_tensor("wmlp1", [KF, P, KD, P], bf16, kind="ExternalInput")
    bmlp1_col_d = nc.dram_tensor("bmlp1_col", [P, KF], f32, kind="ExternalInput")
    wmlp2_d = nc.dram_tensor("wmlp2", [KD, P, KF, P], bf16, kind="ExternalInput")
    bmlp2_col_d = nc.dram_tensor("bmlp2_col", [P, KD], f32, kind="ExternalInput")
    wada_d = nc.dram_tensor("wada", [24, P, KD, NCH], bf16, kind="ExternalInput")
    bada_row_d = nc.dram_tensor("bada_row", [1, ADA], bf16, kind="ExternalInput")
    bprojrow_d = nc.dram_tensor("bprojrow", [1, D], bf16, kind="ExternalInput")
    bm2row_d = nc.dram_tensor("bm2row", [1, D], bf16, kind="ExternalInput")
    outT_d = nc.dram_tensor("outT", [D, NTOK], f32, kind="ExternalOutput")

    xT_r = xT_d.rearrange("(mt p) t -> p mt t", p=P)
    outT_r = outT_d.rearrange("(mt p) t -> p mt t", p=P)

    with tile.TileContext(nc) as tc, ExitStack() as ctx:
        persist = ctx.enter_context(tc.tile_pool(name="persist", bufs=1))
        w8 = ctx.enter_context(tc.tile_pool(name="w8", bufs=2))
        tmp = ctx.enter_context(tc.tile_pool(name="tmp", bufs=2))
        rows = ctx.enter_context(tc.tile_pool(name="rows", bufs=1))
        dram = ctx.enter_context(tc.tile_pool(name="drsc", bufs=2, space="DRAM"))

        ones_b = persist.tile([P, 1], bf16)
        nc.vector.memset(ones_b, 1.0)
        onesrow_b = persist.tile([1, P], bf16)
        nc.vector.memset(onesrow_b, 1.0)
        ones2d = persist.tile([P, 512], bf16)
        nc.vector.memset(ones2d, 1.0)
        eps_col = persist.tile([P, 1], f32)
        nc.vector.memset(eps_col, EPS)
        invd2 = persist.tile([1, 2], bf16)
        nc.vector.memset(invd2, 1.0 / D)

        ccol_sb = persist.tile([P, KD], f32)
        nc.sync.dma_start(ccol_sb[:], ccol_d[:])
        csig = persist.tile([P, KD], f32)
        nc.scalar.activation(csig[:], ccol_sb[:], AF.Sigmoid)
        scol = persist.tile([P, KD], bf16)
        nc.vector.tensor_mul(scol[:], ccol_sb[:], csig[:])

        # lnw row0 = [g1_row | g2_row] (filled as mod chunks land), row1 = 1
        # (memset everything to 1; the g copies overwrite row 0 — engines
        # cannot address partition ranges that start at partition 1)
        lnw = persist.tile([2, 2 * D], bf16)
        nc.vector.memset(lnw[:, :], 1.0)

        modcol1 = persist.tile([P, 16], f32)
        modcol2 = persist.tile([P, 32], f32)
        mod_d = dram.tile([1, ADA], f32, tag="modd")

        # ---------------- LN helpers (feature dim = partitions) ----------
        def ln_stats_tile(accs, xb, kt):
            # bf16 stats matmuls off the resident bf16 copy (quantization
            # noise averages out over the 1024-wide sums)
            xsq = tmp.tile([P, NTOK], bf16, tag="lnworkb")
            nc.scalar.activation(xsq[:], xb[:], AF.Square)
            for hi, h0 in enumerate(HALVES):
                nc.tensor.matmul(accs[hi][0:1, :], lhsT=ones_b[:, 0:1],
                                 rhs=xb[:, h0:h0 + 512],
                                 start=(kt == 0), stop=(kt == KD - 1))
                nc.tensor.matmul(accs[2 + hi][0:1, :], lhsT=ones_b[:, 0:1],
                                 rhs=xsq[:, h0:h0 + 512],
                                 start=(kt == 0), stop=(kt == KD - 1))

        def ln_chain(accs, cm_stats, tag):
            """stats psums -> abr [2, 2048] bf16 rows: a=rstd | b=-mu*rstd,
            identical on partitions 0 and 1 (PE broadcast, no DRAM). All f32
            intermediates live in PSUM; the stats pool is closed right after
            the row copies so the chain\'s 8 banks fit."""
            statrow = rows.tile([1, 2 * NTOK], bf16, tag="statrow")
            for j, acc in enumerate(accs):  # s_h0 s_h1 q_h0 q_h1
                nc.vector.tensor_copy(statrow[0:1, j * 512:(j + 1) * 512],
                                      acc[0:1, :])
            cm_stats.__exit__(None, None, None)
            abr = rows.tile([2, 2 * NTOK], bf16, tag="abr")
            # DVE may read at most ONE input from PSUM per op, so mu and one
            # scratch row live in SBUF.
            mu_s = rows.tile([2, NTOK], f32, tag="cmu")
            w1 = rows.tile([2, NTOK], f32, tag="cw1")
            with tc.tile_pool(name="bc" + tag, bufs=1, space="PSUM") as bc:
                mu2 = bc.tile([2, NTOK], f32, tag="mu2")
                msq2 = bc.tile([2, NTOK], f32, tag="msq2")
                tv = bc.tile([2, NTOK], f32, tag="tv")
                for h in range(2):
                    nc.tensor.matmul(mu2[:, h * 512:(h + 1) * 512],
                                     lhsT=invd2[0:1, :],
                                     rhs=statrow[0:1, h * 512:(h + 1) * 512],
                                     start=True, stop=True)
                    nc.tensor.matmul(msq2[:, h * 512:(h + 1) * 512],
                                     lhsT=invd2[0:1, :],
                                     rhs=statrow[0:1, NTOK + h * 512:NTOK + (h + 1) * 512],
                                     start=True, stop=True)
                nc.vector.tensor_copy(mu_s[:], mu2[:])
                nc.vector.tensor_mul(w1[:], mu_s[:], mu_s[:])
                nc.vector.tensor_sub(tv[:], msq2[:], w1[:])       # var
                # rstd = exp(-0.5*ln(var+eps)) — both funcs live in the same
                # ACT table set as the attention exp; no reciprocal anywhere
                nc.scalar.activation(w1[:], tv[:], AF.Ln, bias=eps_col[0:2, 0:1])
                nc.scalar.activation(tv[:], w1[:], AF.Exp, scale=-0.5)
                nc.vector.tensor_copy(abr[:, 0:NTOK], tv[:])
                nc.vector.scalar_tensor_tensor(abr[:, NTOK:], mu_s[:], -1.0,
                                               tv[:], op0=OP.mult, op1=OP.mult)
            return abr

        def ln_apply(xf, abr, goff, becol, out_bf, ab_pool):
            """h = x*A + (beta + B), A/B from K=2 outer-product matmuls.
            goff: 0 for LN1 (g1 in lnw), D for LN2 (g2). The DVE pair for
            iteration k is emitted after iteration k+1's matmuls so the DVE
            stream never idles on PSUM-slot semaphore round trips."""
            def emit_dve(p):
                kt, hs, AB = p
                t1 = tmp.tile([P, 512], f32, tag="lnwork")
                nc.vector.tensor_mul(t1[:, :], xf[:, kt, hs], AB[:, 0:512])
                nc.vector.scalar_tensor_tensor(out_bf[:, kt, hs], t1[:, :],
                                               becol[:, kt:kt + 1],
                                               AB[:, 512:1024],
                                               op0=OP.add, op1=OP.add)

            pend = None
            for kt in range(KD):
                for h0 in HALVES:
                    hs = slice(h0, h0 + 512)
                    AB = ab_pool.tile([P, NTOK], f32, tag="ab")
                    nc.tensor.matmul(AB[:, 0:512],
                                     lhsT=lnw[0:2, goff + kt * P:goff + (kt + 1) * P],
                                     rhs=abr[0:2, h0:h0 + 512],
                                     start=True, stop=True)
                    nc.tensor.matmul(AB[:, 512:1024],
                                     lhsT=lnw[0:2, goff + kt * P:goff + (kt + 1) * P],
                                     rhs=abr[0:2, NTOK + h0:NTOK + h0 + 512],
                                     start=True, stop=True)
                    if pend is not None:
                        emit_dve(pend)
                    pend = (kt, hs, AB)
            emit_dve(pend)

        # ---------------- adaLN modulation chunk ----------------
        def mod_chunk(chk, pspool, wpool):
            sl = slice(chk * NCH, (chk + 1) * NCH)
            wada_t = wpool.tile([P, KD, NCH], bf16, tag="wada")
            nc.scalar.dma_start(wada_t[:], wada_d[chk])
            ps = pspool.tile([1, NCH], f32, tag="mod")
            for kt in range(KD):
                nc.tensor.matmul(ps[0:1, :], lhsT=scol[:, kt:kt + 1],
                                 rhs=wada_t[:, kt, :], start=(kt == 0), stop=False)
            bada_t = tmp.tile([1, NCH], bf16, tag="badach")
            nc.sync.dma_start(bada_t[:], bada_row_d[0:1, sl])
            nc.tensor.matmul(ps[0:1, :], lhsT=onesrow_b[0:1, 0:1],
                             rhs=bada_t[0:1, :], start=False, stop=True)
            mr = rows.tile([1, NCH], f32, tag="modr", bufs=2)
            nc.scalar.activation(mr[0:1, :], ps[0:1, :], AF.Copy)
            nc.sync.dma_start(mod_d[0:1, sl], mr[:])
            if 0 <= chk < 4:        # g1 row -> lnw (bf16 cast, no DRAM trip)
                nc.vector.tensor_copy(lnw[0:1, chk * NCH:(chk + 1) * NCH], mr[:])
            elif 12 <= chk < 16:    # g2 row -> lnw
                nc.vector.tensor_copy(
                    lnw[0:1, D + (chk - 12) * NCH:D + (chk - 11) * NCH], mr[:])

        # SBUF pool stack (pushed in reverse close order):
        #   X1 (x1bf+tproj, to end) < XBF (xbf, to era6) < A (hT/o/bv, era3-6)
        #   < B (qkT/v, era3-5) < short-lived nested pools per era.
        cmX1 = tc.tile_pool(name="x1pool", bufs=1)
        poolX1 = cmX1.__enter__()
        x1bf = poolX1.tile([P, KD, NTOK], bf16, tag="x1bf")
        cmXBF = tc.tile_pool(name="xbfp", bufs=1)
        poolXBF = cmXBF.__enter__()
        xbf = poolXBF.tile([P, KD, NTOK], bf16, tag="xbf")

        # ======== era 1: x stream + LN1 stats + mod chunks 0-7 ========
        cm_st = tc.tile_pool(name="psst", bufs=4, space="PSUM")
        ps_st = cm_st.__enter__()
        ln1_accs = [ps_st.tile([1, 512], f32, tag="st", name=f"st{j}")
                    for j in range(4)]
        # x streamed at half-tile granularity over two queues (finer
        # DMA/compute pipelining); mod chunks 0-7 interleaved so their
        # matmuls fill the x-DMA wait gaps in the PE FIFO
        xq = [nc.sync, nc.gpsimd]
        with tc.tile_pool(name="xstream", bufs=4) as xsp, \
             tc.tile_pool(name="wadapA", bufs=2) as wadaA:
            cm_mod = tc.tile_pool(name="psmod", bufs=2, space="PSUM")
            ps_mod = cm_mod.__enter__()
            for kt in range(KD):
                for hi, h0 in enumerate(HALVES):
                    hs = slice(h0, h0 + 512)
                    xt = xsp.tile([P, 512], f32, tag="xstream")
                    xq[hi].dma_start(xt[:], xT_r[:, kt, hs])
                    nc.vector.tensor_copy(xbf[:, kt, hs], xt[:])
                    xsq = tmp.tile([P, 512], bf16, tag="lnworkb")
                    nc.scalar.activation(xsq[:], xbf[:, kt, hs], AF.Square)
                    nc.tensor.matmul(ln1_accs[hi][0:1, :], lhsT=ones_b[:, 0:1],
                                     rhs=xbf[:, kt, hs],
                                     start=(kt == 0), stop=(kt == KD - 1))
                    nc.tensor.matmul(ln1_accs[2 + hi][0:1, :],
                                     lhsT=ones_b[:, 0:1], rhs=xsq[:],
                                     start=(kt == 0), stop=(kt == KD - 1))
                mod_chunk(kt, ps_mod, wadaA)  # g1 | be1 for LN1 apply
            cm_mod.__exit__(None, None, None)
        nc.sync.dma_start(
            modcol1[:], mod_d[0:1, 0:2 * D].rearrange("o (j p) -> p (o j)", p=P))

        # ======== era 2: LN1 chain ========
        abr1 = ln_chain(ln1_accs, cm_st, "1")
        be1col = modcol1[:, 8:16]

        # ======== era 3: LN1 apply -> hT, qkT (+mod 8-23), v ========
        cmA = tc.tile_pool(name="attnA", bufs=1)
        attnA = cmA.__enter__()
        cmB = tc.tile_pool(name="attnB", bufs=1)
        attnB = cmB.__enter__()

        hT = attnA.tile([P, KD, NTOK], bf16, tag="hT")
        cm_qv = tc.tile_pool(name="psqv", bufs=2, space="PSUM")
        ps_qv = cm_qv.__enter__()
        cm_ab = tc.tile_pool(name="psab", bufs=2, space="PSUM")
        ps_ab = cm_ab.__enter__()
        ln_apply(xbf, abr1, 0, be1col, hT, ps_ab)
        cm_ab.__exit__(None, None, None)

        bqk_sb = persist.tile([P, 16], f32)
        nc.sync.dma_start(bqk_sb[:], bqk_col_d[:])
        bv_sb = attnA.tile([1, D], bf16, tag="bv")
        nc.sync.dma_start(bv_sb[:], bv_row_d[:])

        qkT = attnB.tile([P, 16, NTOK], bf16, tag="qkT")
        cm_wvp = tc.tile_pool(name="wvp", bufs=2)
        wvp = cm_wvp.__enter__()
        wvhs = []
        for hv, h0 in enumerate(HALVES):
            wvh = wvp.tile([P, KD, 512], bf16, tag="wvh")
            nc.scalar.dma_start(wvh[:], wv_d[:, :, h0:h0 + 512])
            wvhs.append(wvh)
        cm_wadaB = tc.tile_pool(name="wadapB", bufs=1)
        wadaB = cm_wadaB.__enter__()
        cm_mod2 = tc.tile_pool(name="psmod2", bufs=2, space="PSUM")
        ps_mod2 = cm_mod2.__enter__()
        for mt in range(16):
            wt = w8.tile([P, KD, P], bf16, tag="w8")
            nc.sync.dma_start(wt[:], wqk_d[mt])
            ps = ps_qv.tile([P, NTOK], f32, tag="qv")
            for h0 in HALVES:
                for kt in range(KD):
                    nc.tensor.matmul(ps[:, h0:h0 + 512], lhsT=wt[:, kt, :],
                                     rhs=hT[:, kt, h0:h0 + 512],
                                     start=(kt == 0), stop=(kt == KD - 1))
            # bias add on ACT (Identity w/ per-partition bias), frees the DVE
            nc.scalar.activation(qkT[:, mt, :], ps[:, :], AF.Identity,
                                 bias=bqk_sb[:, mt:mt + 1])
            mod_chunk(8 + mt, ps_mod2, wadaB)  # chunks 8..23 in qkT PE slack
        nc.sync.dma_start(
            modcol2[:], mod_d[0:1, 2 * D:6 * D].rearrange("o (j p) -> p (o j)", p=P))
        cm_mod2.__exit__(None, None, None)
        cm_wadaB.__exit__(None, None, None)

        # v GEMM: [1024 tok, 1024 vdims], augmented with a ones column.
        # wv streamed one vdim-half at a time to halve its SBUF footprint.
        v_sb = attnB.tile([P, KD, H, DH + 1], bf16, tag="v")
        nc.vector.memset(v_sb[:, :, :, DH:DH + 1], 1.0)
        cm_psv = tc.tile_pool(name="psv", bufs=2, space="PSUM")
        ps_v = cm_psv.__enter__()
        for hv, h0 in enumerate(HALVES):
            wvh = wvhs[hv]
            for mt in range(KD):  # token tiles
                ps = ps_v.tile([P, 512], f32, tag="vh")
                for kt in range(KD):
                    nc.tensor.matmul(ps[:, :],
                                     lhsT=hT[:, kt, mt * P:(mt + 1) * P],
                                     rhs=wvh[:, kt, :],
                                     start=(kt == 0), stop=False)
                nc.tensor.matmul(ps[:, :], lhsT=onesrow_b[0:1, :],
                                 rhs=bv_sb[0:1, h0:h0 + 512],
                                 start=False, stop=True)
                nc.scalar.activation(
                    v_sb[:, mt, hv * 8:(hv + 1) * 8, 0:DH],
                    ps.rearrange("p (h d) -> p h d", h=8), AF.Copy)
        cm_wvp.__exit__(None, None, None)

        # ======== era 4: attention (exp-bound) ========
        cm_psv.__exit__(None, None, None)
        cm_qv.__exit__(None, None, None)
        cm_sc = tc.tile_pool(name="pssc", bufs=2, space="PSUM")
        ps_sc = cm_sc.__enter__()
        cm_oa = tc.tile_pool(name="psoa", bufs=4, space="PSUM")
        ps_oa = cm_oa.__enter__()
        cm_eb = tc.tile_pool(name="ebuf", bufs=2)
        ebuf = cm_eb.__enter__()
        cm_zp = tc.tile_pool(name="zp", bufs=1)
        zpool = cm_zp.__enter__()

        o_sb = attnA.tile([P, KD, NTOK], bf16, tag="o")
        scale = DH ** -0.5
        a1col = modcol2[:, 0:8]
        be2col = modcol2[:, 16:24]
        a2col = modcol2[:, 24:32]

        def drain_pe(prev):
            # PE broadcast of both Z rows into one [128,512] region of a
            # rotating sc-pool tile (head1 via a partition-64 row group), a
            # quick copy to SBUF (so the sc slot frees fast), ONE full-width
            # reciprocal, then normalize straight out of the oacc PSUM tiles.
            oaccs, hp, h0, zrow = prev
            zt = ps_sc.tile([P, NTOK], f32, tag="sc")
            nc.tensor.matmul(zt[0:DH, 0:512], lhsT=onesrow_b[0:1, 0:DH],
                             rhs=zrow[0:1, :], start=True, stop=True)
            nc.tensor.matmul(zt[DH:P, 0:512], lhsT=ones2d[DH:DH + 1, 0:DH],
                             rhs=zrow[DH:DH + 1, :], start=True, stop=True)
            zraw = zpool.tile([P, 512], f32, tag="zraw", bufs=1)
            nc.vector.tensor_copy(zraw[:], zt[0:P, 0:512])
            zinv = zpool.tile([P, 512], f32, tag="zinv", bufs=1)
            nc.vector.reciprocal(zinv[:], zraw[:])
            zsh = zpool.tile([DH, 512], f32, tag="zsh", bufs=1)
            nc.vector.tensor_copy(zsh[:], zinv[DH:P, :])
            nc.vector.tensor_mul(o_sb[0:DH, hp, h0:h0 + 512],
                                 oaccs[0][0:DH, :], zinv[0:DH, :])
            nc.vector.tensor_mul(o_sb[DH:P, hp, h0:h0 + 512],
                                 oaccs[1][0:DH, :], zsh[:])

        units = [(hp, h0) for hp in range(8) for h0 in HALVES]
        prev = None
        for it, (hp, h0) in enumerate(units):
            qtile, ktile = hp, 8 + hp
            oaccs = [ps_oa.tile([DH + 1, 512], f32, tag="oacc", name=f"oacc{e}")
                     for e in range(2)]
            def emit_scores(kt):
                ks = slice(kt * P, (kt + 1) * P)
                sc = ps_sc.tile([P, NTOK], f32, tag="sc")
                for e in range(2):
                    pb = e * DH
                    nc.tensor.matmul(sc[:, e * 512:e * 512 + 512],
                                     lhsT=qkT[pb:pb + DH, ktile, ks],
                                     rhs=qkT[pb:pb + DH, qtile, h0:h0 + 512],
                                     start=True, stop=True)
                return sc

            # scores emitted one kt ahead of oV so the PE FIFO always has a
            # scores pair ready while exp(kt) runs (exp stays the pacer)
            scs = emit_scores(0)
            for kt in range(KD):
                et = ebuf.tile([P, NTOK], bf16, tag="e")
                nc.scalar.activation(et[:], scs[:, :], AF.Exp, scale=scale)
                if kt + 1 < KD:
                    scs = emit_scores(kt + 1)
                for e in range(2):
                    nc.tensor.matmul(oaccs[e][0:DH + 1, :],
                                     lhsT=v_sb[:, kt, 2 * hp + e, :],
                                     rhs=et[:, e * 512:e * 512 + 512],
                                     start=(kt == 0), stop=(kt == KD - 1))
                if kt == 1 and prev is not None:
                    drain_pe(prev)   # PE slot behind this unit\'s early scores
                    prev = None
            # copy the Z rows out now (DVE); the PE broadcast + normalize is
            # deferred into the next unit so it never stalls the exp cadence
            zrow = zpool.tile([P, 512], bf16, tag="zrow", bufs=2)
            nc.vector.tensor_copy(zrow[0:1, :], oaccs[0][DH:DH + 1, :])
            nc.vector.tensor_copy(zrow[DH:DH + 1, :], oaccs[1][DH:DH + 1, :])
            prev = (oaccs, hp, h0, zrow)
        drain_pe(prev)

        # ======== era 5: proj + residual -> x1 (SBUF), LN2 stats fused ====
        cm_zp.__exit__(None, None, None)
        cm_eb.__exit__(None, None, None)
        cm_oa.__exit__(None, None, None)
        cm_sc.__exit__(None, None, None)
        cmB.__exit__(None, None, None)   # qkT, v

        cm_st2 = tc.tile_pool(name="psst2", bufs=4, space="PSUM")
        ps_st2 = cm_st2.__enter__()
        cm_pp = tc.tile_pool(name="pspp", bufs=2, space="PSUM")
        ps_pp = cm_pp.__enter__()

        bproj_sb = persist.tile([P, KD], f32)
        nc.sync.dma_start(bproj_sb[:], bproj_col_d[:])
        ln2_accs = [ps_st2.tile([1, 512], f32, tag="st2", name=f"st2_{j}")
                    for j in range(4)]
        for mt in range(KD):
            wt = w8.tile([P, KD, P], bf16, tag="w8")
            nc.sync.dma_start(wt[:], wproj_d[mt])
            ps = ps_pp.tile([P, NTOK], f32, tag="pp")
            for h0 in HALVES:
                for kt in range(KD):
                    nc.tensor.matmul(ps[:, h0:h0 + 512], lhsT=wt[:, kt, :],
                                     rhs=o_sb[:, kt, h0:h0 + 512],
                                     start=(kt == 0), stop=(kt == KD - 1))
            tp = poolX1.tile([P, NTOK], f32, tag="tproj", bufs=2)
            nc.vector.tensor_scalar(out=tp[:, :], in0=ps[:, :],
                                    scalar1=bproj_sb[:, mt:mt + 1],
                                    scalar2=a1col[:, mt:mt + 1],
                                    op0=OP.add, op1=OP.mult)
            # residual add straight to bf16 (no separate ACT recast; keeps
            # the ACT queue clear so the LN2 chain starts as early as possible)
            nc.vector.tensor_add(x1bf[:, mt, :], tp[:], xbf[:, mt, :])
            ln_stats_tile(ln2_accs, x1bf[:, mt], mt)

        # ======== era 6: LN2 chain ========
        cm_pp.__exit__(None, None, None)
        cmA.__exit__(None, None, None)   # hT, o, bv
        cmXBF.__exit__(None, None, None)  # xbf
        abr2 = ln_chain(ln2_accs, cm_st2, "2")

        # ======== era 7: LN2 apply -> h2, single-pass MLP ========
        with tc.tile_pool(name="mlp", bufs=1) as mlp, \
             tc.tile_pool(name="w32", bufs=2) as w32:
            h2T = mlp.tile([P, KD, NTOK], bf16, tag="h2T")
            cm_psx = tc.tile_pool(name="psx", bufs=2, space="PSUM")
            ps_x = cm_psx.__enter__()
            cm_ab2 = tc.tile_pool(name="psab2", bufs=2, space="PSUM")
            ps_ab2 = cm_ab2.__enter__()

            bm1_sb = persist.tile([P, KF], f32)
            nc.sync.dma_start(bm1_sb[:], bmlp1_col_d[:])
            bm2_sb = persist.tile([P, KD], f32)
            nc.sync.dma_start(bm2_sb[:], bmlp2_col_d[:])
            m1 = mlp.tile([P, KF, NTOK], bf16, tag="m1")

            # LN2 apply with mlp1's mt=0 matmuls fused into the pipeline:
            # the PE consumes each h2 tile the moment its stt lands, so
            # gelu(mt=0) fires right at apply end instead of a full mt later.
            wt0 = w8.tile([P, KD, P], bf16, tag="w8")
            nc.scalar.dma_start(wt0[:], wmlp1_d[0])
            ps0 = ps_x.tile([P, NTOK], f32, tag="mmx")

            def dve_and_mm(p):
                kt, h0, AB = p
                hs = slice(h0, h0 + 512)
                t1 = tmp.tile([P, 512], f32, tag="lnwork")
                nc.vector.tensor_mul(t1[:, :], x1bf[:, kt, hs], AB[:, 0:512])
                nc.vector.scalar_tensor_tensor(h2T[:, kt, hs], t1[:, :],
                                               be2col[:, kt:kt + 1],
                                               AB[:, 512:1024],
                                               op0=OP.add, op1=OP.add)
                nc.tensor.matmul(ps0[:, hs], lhsT=wt0[:, kt, :],
                                 rhs=h2T[:, kt, hs],
                                 start=(kt == 0), stop=(kt == KD - 1))

            pend = None
            for kt in range(KD):
                for h0 in HALVES:
                    AB = ps_ab2.tile([P, NTOK], f32, tag="ab")
                    nc.tensor.matmul(AB[:, 0:512],
                                     lhsT=lnw[0:2, D + kt * P:D + (kt + 1) * P],
                                     rhs=abr2[0:2, h0:h0 + 512],
                                     start=True, stop=True)
                    nc.tensor.matmul(AB[:, 512:1024],
                                     lhsT=lnw[0:2, D + kt * P:D + (kt + 1) * P],
                                     rhs=abr2[0:2, NTOK + h0:NTOK + h0 + 512],
                                     start=True, stop=True)
                    if pend is not None:
                        dve_and_mm(pend)
                    pend = (kt, h0, AB)
            dve_and_mm(pend)
            cm_ab2.__exit__(None, None, None)
            nc.scalar.activation(m1[:, 0, :], ps0[:, :], AF.Gelu,
                                 bias=bm1_sb[:, 0:1])

            for mt in range(1, KF):
                wt = w8.tile([P, KD, P], bf16, tag="w8")
                nc.scalar.dma_start(wt[:], wmlp1_d[mt])
                ps = ps_x.tile([P, NTOK], f32, tag="mmx")
                for h0 in HALVES:
                    for kt in range(KD):
                        nc.tensor.matmul(ps[:, h0:h0 + 512], lhsT=wt[:, kt, :],
                                         rhs=h2T[:, kt, h0:h0 + 512],
                                         start=(kt == 0), stop=(kt == KD - 1))
                nc.scalar.activation(m1[:, mt, :], ps[:, :], AF.Gelu,
                                     bias=bm1_sb[:, mt:mt + 1])
            for mt in range(KD):
                wt = w32.tile([P, KF, P], bf16, tag="w32")
                nc.sync.dma_start(wt[:], wmlp2_d[mt])
                ps = ps_x.tile([P, NTOK], f32, tag="mmx")
                for h0 in HALVES:
                    for kt in range(KF):
                        nc.tensor.matmul(ps[:, h0:h0 + 512], lhsT=wt[:, kt, :],
                                         rhs=m1[:, kt, h0:h0 + 512],
                                         start=(kt == 0), stop=(kt == KF - 1))
                tp = poolX1.tile([P, NTOK], f32, tag="tproj", bufs=2)
                nc.vector.tensor_scalar(out=tp[:, :], in0=ps[:, :],
                                        scalar1=bm2_sb[:, mt:mt + 1],
                                        scalar2=a2col[:, mt:mt + 1],
                                        op0=OP.add, op1=OP.mult)
                nc.vector.tensor_add(tp[:, :], tp[:, :], x1bf[:, mt, :])
                nc.sync.dma_start(outT_r[:, mt, :], tp[:, :])
            cm_psx.__exit__(None, None, None)
        cmX1.__exit__(None, None, None)

    if split_waits:
        _split_multi_waits(nc)
    nc.finalize()
    return nc


def make_in_maps(x, c, w_qkv, b_qkv, w_proj, b_proj, w_mlp1, b_mlp1,
                 w_mlp2, b_mlp2, w_ada, b_ada):
    bf = ml_dtypes.bfloat16

    def blk(w, n_mt):
        # [K, M] -> [mt, p, kt, M//n_mt] contiguous per-M-tile blocks
        K, M = w.shape
        return np.ascontiguousarray(
            np.asarray(w).astype(bf).reshape(K // P, P, n_mt, M // n_mt)
            .transpose(2, 1, 0, 3))

    wqkv = np.asarray(w_qkv)
    shared = {
        "wqk": blk(wqkv[:, :2 * D], 16),
        "wv": np.ascontiguousarray(
            wqkv[:, 2 * D:].astype(bf).reshape(KD, P, D).transpose(1, 0, 2)),
        "bqk_col": np.ascontiguousarray(
            np.asarray(b_qkv)[:2 * D].astype(np.float32).reshape(16, P).T),
        "bv_row": np.ascontiguousarray(
            np.asarray(b_qkv)[2 * D:].astype(bf).reshape(1, D)),
        "wproj": blk(np.asarray(w_proj), KD),
        "bproj_col": np.ascontiguousarray(
            np.asarray(b_proj).astype(np.float32).reshape(KD, P).T),
        "wmlp1": blk(np.asarray(w_mlp1), KF),
        "bmlp1_col": np.ascontiguousarray(
            np.asarray(b_mlp1).astype(np.float32).reshape(KF, P).T),
        "wmlp2": blk(np.asarray(w_mlp2), KD),
        "bmlp2_col": np.ascontiguousarray(
            np.asarray(b_mlp2).astype(np.float32).reshape(KD, P).T),
        "wada": blk(np.asarray(w_ada), 24),
        "bada_row": np.ascontiguousarray(
            np.asarray(b_ada).astype(bf).reshape(1, ADA)),
        "bprojrow": np.ascontiguousarray(
            np.asarray(b_proj).astype(bf).reshape(1, D)),
        "bm2row": np.ascontiguousarray(
            np.asarray(b_mlp2).astype(bf).reshape(1, D)),
    }
    in_maps = []
    for b in range(NCORES):
        m = dict(shared)
        m["xT"] = np.ascontiguousarray(np.asarray(x[b], dtype=np.float32).T)
        m["ccol"] = np.ascontiguousarray(
            np.asarray(c[b], dtype=np.float32).reshape(KD, P).T)
        in_maps.append(m)
    return in_maps


_NC_CACHE = None


def kernel(x, c, w_qkv, b_qkv, w_proj, b_proj, w_mlp1, b_mlp1,
           w_mlp2, b_mlp2, w_ada, b_ada, _trace=False, **_trace_kw):
    global _NC_CACHE
    from concourse.bass_utils import run_bass_kernel_spmd

    x = np.asarray(x)
    if _NC_CACHE is None:
        _NC_CACHE = build_nc()
    nc = _NC_CACHE
    in_maps = make_in_maps(x, c, w_qkv, b_qkv, w_proj, b_proj, w_mlp1, b_mlp1,
                           w_mlp2, b_mlp2, w_ada, b_ada)
    res = run_bass_kernel_spmd(nc, in_maps, core_ids=list(range(NCORES)),
                               trace=_trace, **_trace_kw)
    out = np.stack([res.results[b]["outT"].T for b in range(NCORES)])
    kernel.last_results = res
    return out.astype(np.float32)


# revision 34
# speedup vs baseline: 1.0122x; 1.0122x over previous
"""DiT block kernel for Trainium2, data-parallel over batch across 8 NeuronCores.

Per-core layout: all activations are kept transposed ([feature, token]) so that
every GEMM consumes weights in their native [in, out] layout as lhsT and
activations as rhs, with no on-device transposes anywhere.

v3 notes (from the v2 HW profile: 943us, PE cold 41% of the time, ~7.5us
PE+ACT stall per attention unit on the 1/Z DRAM bounce, 73us hole at the
LN2->MLP seam, MLP weights streamed twice). Measured 673us after:
  - Every partition-broadcast that used to bounce rows through DRAM (softmax
    1/Z, LN rstd) is now a ones-matmul on the PE: copy the PSUM row to SBUF,
    one K=1 matmul broadcasts it across partitions, full-width DVE reciprocal,
    and the normalize multiply reads the oacc PSUM tile directly (which also
    frees the PSUM slot). Zero DMA hops on any per-unit critical path.
  - LN gamma/beta modulation is folded into per-kt outer-product matmuls:
    A = (1+g) (x) rstd and B = (1+g) (x) (-mu*rstd) come from K=2 matmuls
    (lhsT = [g_row; ones]), so the apply is 2 DVE ops per (kt, half).
  - The attention drain for unit n is emitted after unit n+1's first scores
    matmuls so the PE FIFO never blocks the exp cadence (ACT is the bound
    engine in attention); the adaLN chunks 8-23 fill the remaining PE slack
    inside the attention phase instead of lengthening the qkT phase.
  - MLP runs single-pass over the full 1024-token width (hidden [128,32,1024]
    in SBUF) so wmlp1/wmlp2 stream once, not twice.
  - x and x1 stay resident in SBUF as bf16; no x re-read, no x1 DRAM round
    trip. LN rstd comes from exp(-0.5*ln(var+eps)) on the ACT engine (this
    walrus has no fast reciprocal path; DVE reciprocal costs ~6.5ns/elem, so
    the only one left runs once per attention unit on a partition-packed
    [128,512] tile). Attention is exp-bound (ACT ~100%) and the MLP runs at
    the bf16 matmul floor.

Host side shards B=8 one element per core, pre-transposes x, pre-casts weights
to bf16 (fp32 accumulation in PSUM throughout), and transposes the per-core
[D, N] outputs back.
"""
import sys

for _p in ("/opt/trn_rl_repo",):
    if _p not in sys.path:
        sys.path.insert(0, _p)

import numpy as np
import ml_dtypes
from contextlib import ExitStack

import concourse.bass as bass
import concourse.mybir as mybir
import concourse.tile as tile

f32 = mybir.dt.float32
bf16 = mybir.dt.bfloat16
AF = mybir.ActivationFunctionType
OP = mybir.AluOpType

P = 128
NTOK = 1024     # tokens per batch element
D = 1024        # model dim
KD = D // P     # 8 k-tiles over model dim
H = 16          # heads
DH = 64         # head dim
F = 4096        # mlp hidden
KF = F // P     # 32
ADA = 6 * D     # 6144
EPS = 1e-6
NCORES = 8
HALVES = (0, 512)
NCH = 256       # adaLN chunk width


def _split_multi_waits(nc):
    """This container's walrus build encodes at most ONE sync wait per
    instruction ("Too many sync wait commands"); hoist extra waits onto
    single-wait NoOps in the same engine stream."""
    for fn in nc.m.functions:
        for blk in fn.blocks:
            out = []
            for inst in blk.instructions:
                si = inst.sync_info
                waits = list(si.on_wait) if si is not None and si.on_wait else []
                if len(waits) > 1:
                    for i, w in enumerate(waits[:-1]):
                        nop = mybir.InstNoOp(name=f"{inst.name}-ws{i}", ins=[], outs=[])
                        nop.engine = inst.engine
                        nop.sync_info = mybir.SyncInfo(on_wait=[w], on_update=[])
                        out.append(nop)
                    inst.sync_info = mybir.SyncInfo(on_wait=[waits[-1]],
                                                    on_update=list(si.on_update))
                out.append(inst)
            blk.instructions = out


def build_nc(split_waits=True):
    nc = bass.Bass(trn_type="TRN2")

    xT_d = nc.dram_tensor("xT", [D, NTOK], f32, kind="ExternalInput")
    ccol_d = nc.dram_tensor("ccol", [P, KD], f32, kind="ExternalInput")
    wqk_d = nc.dram_tensor("wqk", [16, P, KD, P], bf16, kind="ExternalInput")
    wv_d = nc.dram_tensor("wv", [P, KD, D], bf16, kind="ExternalInput")
    bqk_col_d = nc.dram_tensor("bqk_col", [P, 16], f32, kind="ExternalInput")
    bv_row_d = nc.dram_tensor("bv_row", [1, D], bf16, kind="ExternalInput")
    wproj_d = nc.dram_tensor("wproj", [KD, P, KD, P], bf16, kind="ExternalInput")
    bproj_col_d = nc.dram_tensor("bproj_col", [P, KD], f32, kind="ExternalInput")
    wmlp1_d = nc.dram_tensor("wmlp1", [KF, P, KD, P], bf16, kind="ExternalInput")
    bmlp1_col_d = nc.dram_tensor("bmlp1_col", [P, KF], f32, kind="ExternalInput")
    wmlp2_d = nc.dram_tensor("wmlp2", [KD, P, KF, P], bf16, kind="ExternalInput")
    bmlp2_col_d = nc.dram_tensor("bmlp2_col", [P, KD], f32, kind="ExternalInput")
    wada_d = nc.dram_tensor("wada", [24, P, KD, NCH], bf16, kind="ExternalInput")
    bada_row_d = nc.dram_tensor("bada_row", [1, ADA], bf16, kind="ExternalInput")
    bprojrow_d = nc.dram_tensor("bprojrow", [1, D], bf16, kind="ExternalInput")
    bm2row_d = nc.dram_tensor("bm2row", [1, D], bf16, kind="ExternalInput")
    outT_d = nc.dram_tensor("outT", [D, NTOK], f32, kind="ExternalOutput")

    xT_r = xT_d.rearrange("(mt p) t -> p mt t", p=P)
    outT_r = outT_d.rearrange("(mt p) t -> p mt t", p=P)

    with tile.TileContext(nc) as tc, ExitStack() as ctx:
        persist = ctx.enter_context(tc.tile_pool(name="persist", bufs=1))
        w8 = ctx.enter_context(tc.tile_pool(name="w8", bufs=2))
        tmp = ctx.enter_context(tc.tile_pool(name="tmp", bufs=2))
        rows = ctx.enter_context(tc.tile_pool(name="rows", bufs=1))
        dram = ctx.enter_context(tc.tile_pool(name="drsc", bufs=2, space="DRAM"))

        ones_b = persist.tile([P, 1], bf16)
        nc.vector.memset(ones_b, 1.0)
        onesrow_b = persist.tile([1, P], bf16)
        nc.vector.memset(onesrow_b, 1.0)
        ones2d = persist.tile([P, 512], bf16)
        nc.vector.memset(ones2d, 1.0)
        eps_col = persist.tile([P, 1], f32)
        nc.vector.memset(eps_col, EPS)
        invd2 = persist.tile([1, 2], bf16)
        nc.vector.memset(invd2, 1.0 / D)

        ccol_sb = persist.tile([P, KD], f32)
        nc.sync.dma_start(ccol_sb[:], ccol_d[:])
        csig = persist.tile([P, KD], f32)
        nc.scalar.activation(csig[:], ccol_sb[:], AF.Sigmoid)
        scol = persist.tile([P, KD], bf16)
        nc.vector.tensor_mul(scol[:], ccol_sb[:], csig[:])

        # lnw row0 = [g1_row | g2_row] (filled as mod chunks land), row1 = 1
        # (memset everything to 1; the g copies overwrite row 0 — engines
        # cannot address partition ranges that start at partition 1)
        lnw = persist.tile([2, 2 * D], bf16)
        nc.vector.memset(lnw[:, :], 1.0)

        modcol1 = persist.tile([P, 16], f32)
        modcol2 = persist.tile([P, 32], f32)
        mod_d = dram.tile([1, ADA], f32, tag="modd")

        # ---------------- LN helpers (feature dim = partitions) ----------
        def ln_stats_tile(accs, xb, kt):
            # bf16 stats matmuls off the resident bf16 copy (quantization
            # noise averages out over the 1024-wide sums)
            xsq = tmp.tile([P, NTOK], bf16, tag="lnworkb")
            nc.scalar.activation(xsq[:], xb[:], AF.Square)
            for hi, h0 in enumerate(HALVES):
                nc.tensor.matmul(accs[hi][0:1, :], lhsT=ones_b[:, 0:1],
                                 rhs=xb[:, h0:h0 + 512],
                                 start=(kt == 0), stop=(kt == KD - 1))
                nc.tensor.matmul(accs[2 + hi][0:1, :], lhsT=ones_b[:, 0:1],
                                 rhs=xsq[:, h0:h0 + 512],
                                 start=(kt == 0), stop=(kt == KD - 1))

        def ln_chain(accs, cm_stats, tag):
            """stats psums -> abr [2, 2048] bf16 rows: a=rstd | b=-mu*rstd,
            identical on partitions 0 and 1 (PE broadcast, no DRAM). All f32
            intermediates live in PSUM; the stats pool is closed right after
            the row copies so the chain\'s 8 banks fit."""
            statrow = rows.tile([1, 2 * NTOK], bf16, tag="statrow")
            for j, acc in enumerate(accs):  # s_h0 s_h1 q_h0 q_h1
                # split the row copies across DVE and ACT so they run in
                # parallel (both engines are idle at chain time)
                if j < 2:
                    nc.vector.tensor_copy(statrow[0:1, j * 512:(j + 1) * 512],
                                          acc[0:1, :])
                else:
                    nc.scalar.copy(statrow[0:1, j * 512:(j + 1) * 512],
                                   acc[0:1, :])
            cm_stats.__exit__(None, None, None)
            abr = rows.tile([2, 2 * NTOK], bf16, tag="abr")
            w1 = rows.tile([2, NTOK], f32, tag="cw1")
            with tc.tile_pool(name="bc" + tag, bufs=1, space="PSUM") as bc:
                mu2 = bc.tile([2, NTOK], f32, tag="mu2")
                msq2 = bc.tile([2, NTOK], f32, tag="msq2")
                tv = bc.tile([2, NTOK], f32, tag="tv")
                for h in range(2):
                    nc.tensor.matmul(mu2[:, h * 512:(h + 1) * 512],
                                     lhsT=invd2[0:1, :],
                                     rhs=statrow[0:1, h * 512:(h + 1) * 512],
                                     start=True, stop=True)
                    nc.tensor.matmul(msq2[:, h * 512:(h + 1) * 512],
                                     lhsT=invd2[0:1, :],
                                     rhs=statrow[0:1, NTOK + h * 512:NTOK + (h + 1) * 512],
                                     start=True, stop=True)
                # mu^2 on the idle ACT straight off PSUM (drops the mu SBUF
                # copy from the serial chain); var = E[x^2] - mu^2; then
                # rstd = exp(-0.5*ln(var+eps)) — square/ln/exp all live in
                # one ACT table set, and no reciprocal anywhere
                nc.scalar.activation(w1[:], mu2[:], AF.Square)
                nc.vector.tensor_sub(tv[:], msq2[:], w1[:])       # var
                nc.scalar.activation(w1[:], tv[:], AF.Ln, bias=eps_col[0:2, 0:1])
                nc.scalar.activation(tv[:], w1[:], AF.Exp, scale=-0.5)
                nc.vector.tensor_copy(abr[:, 0:NTOK], tv[:])
                # b = -mu * rstd, reading mu from PSUM and the just-cast bf16
                # rstd row as the SBUF operand (one-PSUM-input rule)
                nc.vector.scalar_tensor_tensor(abr[:, NTOK:], mu2[:], -1.0,
                                               abr[:, 0:NTOK],
                                               op0=OP.mult, op1=OP.mult)
            return abr

        def ln_apply(xf, abr, goff, becol, out_bf, ab_pool):
            """h = x*A + (beta + B), A/B from K=2 outer-product matmuls.
            goff: 0 for LN1 (g1 in lnw), D for LN2 (g2). The DVE pair for
            iteration k is emitted after iteration k+1's matmuls so the DVE
            stream never idles on PSUM-slot semaphore round trips."""
            def emit_dve(p):
                kt, hs, AB = p
                t1 = tmp.tile([P, 512], f32, tag="lnwork")
                nc.vector.tensor_mul(t1[:, :], xf[:, kt, hs], AB[:, 0:512])
                nc.vector.scalar_tensor_tensor(out_bf[:, kt, hs], t1[:, :],
                                               becol[:, kt:kt + 1],
                                               AB[:, 512:1024],
                                               op0=OP.add, op1=OP.add)

            pend = None
            for kt in range(KD):
                for h0 in HALVES:
                    hs = slice(h0, h0 + 512)
                    AB = ab_pool.tile([P, NTOK], f32, tag="ab")
                    nc.tensor.matmul(AB[:, 0:512],
                                     lhsT=lnw[0:2, goff + kt * P:goff + (kt + 1) * P],
                                     rhs=abr[0:2, h0:h0 + 512],
                                     start=True, stop=True)
                    nc.tensor.matmul(AB[:, 512:1024],
                                     lhsT=lnw[0:2, goff + kt * P:goff + (kt + 1) * P],
                                     rhs=abr[0:2, NTOK + h0:NTOK + h0 + 512],
                                     start=True, stop=True)
                    if pend is not None:
                        emit_dve(pend)
                    pend = (kt, hs, AB)
            emit_dve(pend)

        # ---------------- adaLN modulation chunk ----------------
        def mod_chunk(chk, pspool, wpool):
            sl = slice(chk * NCH, (chk + 1) * NCH)
            wada_t = wpool.tile([P, KD, NCH], bf16, tag="wada")
            nc.scalar.dma_start(wada_t[:], wada_d[chk])
            ps = pspool.tile([1, NCH], f32, tag="mod")
            for kt in range(KD):
                nc.tensor.matmul(ps[0:1, :], lhsT=scol[:, kt:kt + 1],
                                 rhs=wada_t[:, kt, :], start=(kt == 0), stop=False)
            bada_t = tmp.tile([1, NCH], bf16, tag="badach")
            nc.sync.dma_start(bada_t[:], bada_row_d[0:1, sl])
            nc.tensor.matmul(ps[0:1, :], lhsT=onesrow_b[0:1, 0:1],
                             rhs=bada_t[0:1, :], start=False, stop=True)
            mr = rows.tile([1, NCH], f32, tag="modr", bufs=2)
            nc.scalar.activation(mr[0:1, :], ps[0:1, :], AF.Copy)
            nc.sync.dma_start(mod_d[0:1, sl], mr[:])
            if 0 <= chk < 4:        # g1 row -> lnw (bf16 cast, no DRAM trip)
                nc.vector.tensor_copy(lnw[0:1, chk * NCH:(chk + 1) * NCH], mr[:])
            elif 12 <= chk < 16:    # g2 row -> lnw
                nc.vector.tensor_copy(
                    lnw[0:1, D + (chk - 12) * NCH:D + (chk - 11) * NCH], mr[:])

        # SBUF pool stack (pushed in reverse close order):
        #   X1 (x1bf+tproj, to end) < XBF (xbf, to era6) < A (hT/o/bv, era3-6)
        #   < B (qkT/v, era3-5) < short-lived nested pools per era.
        cmX1 = tc.tile_pool(name="x1pool", bufs=1)
        poolX1 = cmX1.__enter__()
        x1bf = poolX1.tile([P, KD, NTOK], bf16, tag="x1bf")
        cmXBF = tc.tile_pool(name="xbfp", bufs=1)
        poolXBF = cmXBF.__enter__()
        xbf = poolXBF.tile([P, KD, NTOK], bf16, tag="xbf")

        # ======== era 1: x stream + LN1 stats + mod chunks 0-7 ========
        cm_st = tc.tile_pool(name="psst", bufs=4, space="PSUM")
        ps_st = cm_st.__enter__()
        ln1_accs = [ps_st.tile([1, 512], f32, tag="st", name=f"st{j}")
                    for j in range(4)]
        # x streamed at half-tile granularity over two queues (finer
        # DMA/compute pipelining); mod chunks 0-7 interleaved so their
        # matmuls fill the x-DMA wait gaps in the PE FIFO
        xq = [nc.sync, nc.gpsimd]
        with tc.tile_pool(name="xstream", bufs=4) as xsp, \
             tc.tile_pool(name="wadapA", bufs=2) as wadaA:
            cm_mod = tc.tile_pool(name="psmod", bufs=2, space="PSUM")
            ps_mod = cm_mod.__enter__()
            for kt in range(KD):
                for hi, h0 in enumerate(HALVES):
                    hs = slice(h0, h0 + 512)
                    xt = xsp.tile([P, 512], f32, tag="xstream")
                    xq[hi].dma_start(xt[:], xT_r[:, kt, hs])
                    nc.vector.tensor_copy(xbf[:, kt, hs], xt[:])
                    xsq = tmp.tile([P, 512], bf16, tag="lnworkb")
                    nc.scalar.activation(xsq[:], xbf[:, kt, hs], AF.Square)
                    nc.tensor.matmul(ln1_accs[hi][0:1, :], lhsT=ones_b[:, 0:1],
                                     rhs=xbf[:, kt, hs],
                                     start=(kt == 0), stop=(kt == KD - 1))
                    nc.tensor.matmul(ln1_accs[2 + hi][0:1, :],
                                     lhsT=ones_b[:, 0:1], rhs=xsq[:],
                                     start=(kt == 0), stop=(kt == KD - 1))
                mod_chunk(kt, ps_mod, wadaA)  # g1 | be1 for LN1 apply
            cm_mod.__exit__(None, None, None)
        nc.sync.dma_start(
            modcol1[:], mod_d[0:1, 0:2 * D].rearrange("o (j p) -> p (o j)", p=P))

        # ======== era 2: LN1 chain ========
        abr1 = ln_chain(ln1_accs, cm_st, "1")
        be1col = modcol1[:, 8:16]

        # ======== era 3: LN1 apply -> hT, qkT (+mod 8-23), v ========
        cmA = tc.tile_pool(name="attnA", bufs=1)
        attnA = cmA.__enter__()
        cmB = tc.tile_pool(name="attnB", bufs=1)
        attnB = cmB.__enter__()

        hT = attnA.tile([P, KD, NTOK], bf16, tag="hT")
        cm_qv = tc.tile_pool(name="psqv", bufs=2, space="PSUM")
        ps_qv = cm_qv.__enter__()
        cm_ab = tc.tile_pool(name="psab", bufs=2, space="PSUM")
        ps_ab = cm_ab.__enter__()
        ln_apply(xbf, abr1, 0, be1col, hT, ps_ab)
        cm_ab.__exit__(None, None, None)

        bqk_sb = persist.tile([P, 16], f32)
        nc.sync.dma_start(bqk_sb[:], bqk_col_d[:])
        bv_sb = attnA.tile([1, D], bf16, tag="bv")
        nc.sync.dma_start(bv_sb[:], bv_row_d[:])

        qkT = attnB.tile([P, 16, NTOK], bf16, tag="qkT")
        cm_wvp = tc.tile_pool(name="wvp", bufs=2)
        wvp = cm_wvp.__enter__()
        wvhs = []
        for hv, h0 in enumerate(HALVES):
            wvh = wvp.tile([P, KD, 512], bf16, tag="wvh")
            nc.scalar.dma_start(wvh[:], wv_d[:, :, h0:h0 + 512])
            wvhs.append(wvh)
        cm_wadaB = tc.tile_pool(name="wadapB", bufs=1)
        wadaB = cm_wadaB.__enter__()
        cm_mod2 = tc.tile_pool(name="psmod2", bufs=2, space="PSUM")
        ps_mod2 = cm_mod2.__enter__()
        for mt in range(16):
            wt = w8.tile([P, KD, P], bf16, tag="w8")
            nc.sync.dma_start(wt[:], wqk_d[mt])
            ps = ps_qv.tile([P, NTOK], f32, tag="qv")
            for h0 in HALVES:
                for kt in range(KD):
                    nc.tensor.matmul(ps[:, h0:h0 + 512], lhsT=wt[:, kt, :],
                                     rhs=hT[:, kt, h0:h0 + 512],
                                     start=(kt == 0), stop=(kt == KD - 1))
            # bias add on ACT (Identity w/ per-partition bias), frees the DVE
            nc.scalar.activation(qkT[:, mt, :], ps[:, :], AF.Identity,
                                 bias=bqk_sb[:, mt:mt + 1])
            mod_chunk(8 + mt, ps_mod2, wadaB)  # chunks 8..23 in qkT PE slack
        nc.sync.dma_start(
            modcol2[:], mod_d[0:1, 2 * D:6 * D].rearrange("o (j p) -> p (o j)", p=P))
        cm_mod2.__exit__(None, None, None)
        cm_wadaB.__exit__(None, None, None)

        # v GEMM: [1024 tok, 1024 vdims], augmented with a ones column.
        # wv streamed one vdim-half at a time to halve its SBUF footprint.
        v_sb = attnB.tile([P, KD, H, DH + 1], bf16, tag="v")
        nc.vector.memset(v_sb[:, :, :, DH:DH + 1], 1.0)
        cm_psv = tc.tile_pool(name="psv", bufs=2, space="PSUM")
        ps_v = cm_psv.__enter__()
        for hv, h0 in enumerate(HALVES):
            wvh = wvhs[hv]
            for mt in range(KD):  # token tiles
                ps = ps_v.tile([P, 512], f32, tag="vh")
                for kt in range(KD):
                    nc.tensor.matmul(ps[:, :],
                                     lhsT=hT[:, kt, mt * P:(mt + 1) * P],
                                     rhs=wvh[:, kt, :],
                                     start=(kt == 0), stop=False)
                nc.tensor.matmul(ps[:, :], lhsT=onesrow_b[0:1, :],
                                 rhs=bv_sb[0:1, h0:h0 + 512],
                                 start=False, stop=True)
                nc.scalar.activation(
                    v_sb[:, mt, hv * 8:(hv + 1) * 8, 0:DH],
                    ps.rearrange("p (h d) -> p h d", h=8), AF.Copy)
        cm_wvp.__exit__(None, None, None)

        # ======== era 4: attention (exp-bound) ========
        cm_psv.__exit__(None, None, None)
        cm_qv.__exit__(None, None, None)
        cm_sc = tc.tile_pool(name="pssc", bufs=2, space="PSUM")
        ps_sc = cm_sc.__enter__()
        cm_oa = tc.tile_pool(name="psoa", bufs=4, space="PSUM")
        ps_oa = cm_oa.__enter__()
        cm_eb = tc.tile_pool(name="ebuf", bufs=2)
        ebuf = cm_eb.__enter__()
        cm_zp = tc.tile_pool(name="zp", bufs=1)
        zpool = cm_zp.__enter__()

        o_sb = attnA.tile([P, KD, NTOK], bf16, tag="o")
        scale = DH ** -0.5
        a1col = modcol2[:, 0:8]
        be2col = modcol2[:, 16:24]
        a2col = modcol2[:, 24:32]

        def drain_pe(prev):
            # PE broadcast of both Z rows into one [128,512] region of a
            # rotating sc-pool tile (head1 via a partition-64 row group), a
            # quick copy to SBUF (so the sc slot frees fast), ONE full-width
            # reciprocal, then normalize straight out of the oacc PSUM tiles.
            oaccs, hp, h0, zrow = prev
            zt = ps_sc.tile([P, NTOK], f32, tag="sc")
            nc.tensor.matmul(zt[0:DH, 0:512], lhsT=onesrow_b[0:1, 0:DH],
                             rhs=zrow[0:1, :], start=True, stop=True)
            nc.tensor.matmul(zt[DH:P, 0:512], lhsT=ones2d[DH:DH + 1, 0:DH],
                             rhs=zrow[DH:DH + 1, :], start=True, stop=True)
            zraw = zpool.tile([P, 512], f32, tag="zraw", bufs=1)
            nc.vector.tensor_copy(zraw[:], zt[0:P, 0:512])
            zinv = zpool.tile([P, 512], f32, tag="zinv", bufs=1)
            nc.vector.reciprocal(zinv[:], zraw[:])
            zsh = zpool.tile([DH, 512], f32, tag="zsh", bufs=1)
            nc.vector.tensor_copy(zsh[:], zinv[DH:P, :])
            nc.vector.tensor_mul(o_sb[0:DH, hp, h0:h0 + 512],
                                 oaccs[0][0:DH, :], zinv[0:DH, :])
            nc.vector.tensor_mul(o_sb[DH:P, hp, h0:h0 + 512],
                                 oaccs[1][0:DH, :], zsh[:])

        units = [(hp, h0) for hp in range(8) for h0 in HALVES]
        prev = None
        for it, (hp, h0) in enumerate(units):
            qtile, ktile = hp, 8 + hp
            oaccs = [ps_oa.tile([DH + 1, 512], f32, tag="oacc", name=f"oacc{e}")
                     for e in range(2)]
            def emit_scores(kt):
                ks = slice(kt * P, (kt + 1) * P)
                sc = ps_sc.tile([P, NTOK], f32, tag="sc")
                for e in range(2):
                    pb = e * DH
                    nc.tensor.matmul(sc[:, e * 512:e * 512 + 512],
                                     lhsT=qkT[pb:pb + DH, ktile, ks],
                                     rhs=qkT[pb:pb + DH, qtile, h0:h0 + 512],
                                     start=True, stop=True)
                return sc

            # scores emitted one kt ahead of oV so the PE FIFO always has a
            # scores pair ready while exp(kt) runs (exp stays the pacer)
            scs = emit_scores(0)
            for kt in range(KD):
                et = ebuf.tile([P, NTOK], bf16, tag="e")
                nc.scalar.activation(et[:], scs[:, :], AF.Exp, scale=scale)
                if kt + 1 < KD:
                    scs = emit_scores(kt + 1)
                for e in range(2):
                    nc.tensor.matmul(oaccs[e][0:DH + 1, :],
                                     lhsT=v_sb[:, kt, 2 * hp + e, :],
                                     rhs=et[:, e * 512:e * 512 + 512],
                                     start=(kt == 0), stop=(kt == KD - 1))
                if kt == 1 and prev is not None:
                    drain_pe(prev)   # PE slot behind this unit\'s early scores
                    prev = None
            # copy the Z rows out now (DVE); the PE broadcast + normalize is
            # deferred into the next unit so it never stalls the exp cadence
            zrow = zpool.tile([P, 512], bf16, tag="zrow", bufs=2)
            nc.vector.tensor_copy(zrow[0:1, :], oaccs[0][DH:DH + 1, :])
            nc.vector.tensor_copy(zrow[DH:DH + 1, :], oaccs[1][DH:DH + 1, :])
            prev = (oaccs, hp, h0, zrow)
        drain_pe(prev)

        # ======== era 5: proj + residual -> x1 (SBUF), LN2 stats fused ====
        cm_zp.__exit__(None, None, None)
        cm_eb.__exit__(None, None, None)
        cm_oa.__exit__(None, None, None)
        cm_sc.__exit__(None, None, None)
        cmB.__exit__(None, None, None)   # qkT, v

        cm_st2 = tc.tile_pool(name="psst2", bufs=4, space="PSUM")
        ps_st2 = cm_st2.__enter__()
        cm_pp = tc.tile_pool(name="pspp", bufs=2, space="PSUM")
        ps_pp = cm_pp.__enter__()

        bproj_sb = persist.tile([P, KD], f32)
        nc.sync.dma_start(bproj_sb[:], bproj_col_d[:])
        ln2_accs = [ps_st2.tile([1, 512], f32, tag="st2", name=f"st2_{j}")
                    for j in range(4)]
        for mt in range(KD):
            wt = w8.tile([P, KD, P], bf16, tag="w8")
            nc.sync.dma_start(wt[:], wproj_d[mt])
            ps = ps_pp.tile([P, NTOK], f32, tag="pp")
            for h0 in HALVES:
                for kt in range(KD):
                    nc.tensor.matmul(ps[:, h0:h0 + 512], lhsT=wt[:, kt, :],
                                     rhs=o_sb[:, kt, h0:h0 + 512],
                                     start=(kt == 0), stop=(kt == KD - 1))
            tp = poolX1.tile([P, NTOK], f32, tag="tproj", bufs=2)
            nc.vector.tensor_scalar(out=tp[:, :], in0=ps[:, :],
                                    scalar1=bproj_sb[:, mt:mt + 1],
                                    scalar2=a1col[:, mt:mt + 1],
                                    op0=OP.add, op1=OP.mult)
            # residual add straight to bf16 (no separate ACT recast; keeps
            # the ACT queue clear so the LN2 chain starts as early as possible)
            nc.vector.tensor_add(x1bf[:, mt, :], tp[:], xbf[:, mt, :])
            ln_stats_tile(ln2_accs, x1bf[:, mt], mt)

        # ======== era 6: LN2 chain ========
        cm_pp.__exit__(None, None, None)
        cmA.__exit__(None, None, None)   # hT, o, bv
        cmXBF.__exit__(None, None, None)  # xbf
        abr2 = ln_chain(ln2_accs, cm_st2, "2")

        # ======== era 7: LN2 apply -> h2, single-pass MLP ========
        with tc.tile_pool(name="mlp", bufs=1) as mlp, \
             tc.tile_pool(name="w32", bufs=2) as w32:
            h2T = mlp.tile([P, KD, NTOK], bf16, tag="h2T")
            cm_psx = tc.tile_pool(name="psx", bufs=2, space="PSUM")
            ps_x = cm_psx.__enter__()
            cm_ab2 = tc.tile_pool(name="psab2", bufs=2, space="PSUM")
            ps_ab2 = cm_ab2.__enter__()
            ln_apply(x1bf, abr2, D, be2col, h2T, ps_ab2)
            cm_ab2.__exit__(None, None, None)

            bm1_sb = persist.tile([P, KF], f32)
            nc.sync.dma_start(bm1_sb[:], bmlp1_col_d[:])
            bm2_sb = persist.tile([P, KD], f32)
            nc.sync.dma_start(bm2_sb[:], bmlp2_col_d[:])

            m1 = mlp.tile([P, KF, NTOK], bf16, tag="m1")
            for mt in range(KF):
                wt = w8.tile([P, KD, P], bf16, tag="w8")
                nc.scalar.dma_start(wt[:], wmlp1_d[mt])
                ps = ps_x.tile([P, NTOK], f32, tag="mmx")
                for h0 in HALVES:
                    for kt in range(KD):
                        nc.tensor.matmul(ps[:, h0:h0 + 512], lhsT=wt[:, kt, :],
                                         rhs=h2T[:, kt, h0:h0 + 512],
                                         start=(kt == 0), stop=(kt == KD - 1))
                nc.scalar.activation(m1[:, mt, :], ps[:, :], AF.Gelu,
                                     bias=bm1_sb[:, mt:mt + 1])
            for mt in range(KD):
                wt = w32.tile([P, KF, P], bf16, tag="w32")
                nc.sync.dma_start(wt[:], wmlp2_d[mt])
                ps = ps_x.tile([P, NTOK], f32, tag="mmx")
                for h0 in HALVES:
                    for kt in range(KF):
                        nc.tensor.matmul(ps[:, h0:h0 + 512], lhsT=wt[:, kt, :],
                                         rhs=m1[:, kt, h0:h0 + 512],
                                         start=(kt == 0), stop=(kt == KF - 1))
                tp = poolX1.tile([P, NTOK], f32, tag="tproj", bufs=2)
                nc.vector.tensor_scalar(out=tp[:, :], in0=ps[:, :],
                                        scalar1=bm2_sb[:, mt:mt + 1],
                                        scalar2=a2col[:, mt:mt + 1],
                                        op0=OP.add, op1=OP.mult)
                nc.vector.tensor_add(tp[:, :], tp[:, :], x1bf[:, mt, :])
                nc.sync.dma_start(outT_r[:, mt, :], tp[:, :])
            cm_psx.__exit__(None, None, None)
        cmX1.__exit__(None, None, None)

    if split_waits:
        _split_multi_waits(nc)
    nc.finalize()
    return nc


def make_in_maps(x, c, w_qkv, b_qkv, w_proj, b_proj, w_mlp1, b_mlp1,
                 w_mlp2, b_mlp2, w_ada, b_ada):
    bf = ml_dtypes.bfloat16

    def blk(w, n_mt):
        # [K, M] -> [mt, p, kt, M//n_mt] contiguous per-M-tile blocks
        K, M = w.shape
        return np.ascontiguousarray(
            np.asarray(w).astype(bf).reshape(K // P, P, n_mt, M // n_mt)
            .transpose(2, 1, 0, 3))

    wqkv = np.asarray(w_qkv)
    shared = {
        "wqk": blk(wqkv[:, :2 * D], 16),
        "wv": np.ascontiguousarray(
            wqkv[:, 2 * D:].astype(bf).reshape(KD, P, D).transpose(1, 0, 2)),
        "bqk_col": np.ascontiguousarray(
            np.asarray(b_qkv)[:2 * D].astype(np.float32).reshape(16, P).T),
        "bv_row": np.ascontiguousarray(
            np.asarray(b_qkv)[2 * D:].astype(bf).reshape(1, D)),
        "wproj": blk(np.asarray(w_proj), KD),
        "bproj_col": np.ascontiguousarray(
            np.asarray(b_proj).astype(np.float32).reshape(KD, P).T),
        "wmlp1": blk(np.asarray(w_mlp1), KF),
        "bmlp1_col": np.ascontiguousarray(
            np.asarray(b_mlp1).astype(np.float32).reshape(KF, P).T),
        "wmlp2": blk(np.asarray(w_mlp2), KD),
        "bmlp2_col": np.ascontiguousarray(
            np.asarray(b_mlp2).astype(np.float32).reshape(KD, P).T),
        "wada": blk(np.asarray(w_ada), 24),
        "bada_row": np.ascontiguousarray(
            np.asarray(b_ada).astype(bf).reshape(1, ADA)),
        "bprojrow": np.ascontiguousarray(
            np.asarray(b_proj).astype(bf).reshape(1, D)),
        "bm2row": np.ascontiguousarray(
            np.asarray(b_mlp2).astype(bf).reshape(1, D)),
    }
    in_maps = []
    for b in range(NCORES):
        m = dict(shared)
        m["xT"] = np.ascontiguousarray(np.asarray(x[b], dtype=np.float32).T)
        m["ccol"] = np.ascontiguousarray(
            np.asarray(c[b], dtype=np.float32).reshape(KD, P).T)
        in_maps.append(m)
    return in_maps


_NC_CACHE = None


def kernel(x, c, w_qkv, b_qkv, w_proj, b_proj, w_mlp1, b_mlp1,
           w_mlp2, b_mlp2, w_ada, b_ada, _trace=False, **_trace_kw):
    global _NC_CACHE
    from concourse.bass_utils import run_bass_kernel_spmd

    x = np.asarray(x)
    if _NC_CACHE is None:
        _NC_CACHE = build_nc()
    nc = _NC_CACHE
    in_maps = make_in_maps(x, c, w_qkv, b_qkv, w_proj, b_proj, w_mlp1, b_mlp1,
                           w_mlp2, b_mlp2, w_ada, b_ada)
    res = run_bass_kernel_spmd(nc, in_maps, core_ids=list(range(NCORES)),
                               trace=_trace, **_trace_kw)
    out = np.stack([res.results[b]["outT"].T for b in range(NCORES)])
    kernel.last_results = res
    return out.astype(np.float32)
